# revision 57
# baseline (speedup 1.0000x reference)
"""DCVQ quantizer (vq_codebook) on 8 TRN2 NeuronCores.

Sharding (per spec hint): data-parallel over tokens (B*H*W), codebooks
replicated on every core; scalar loss partials summed on host.

Per core pipeline:
  - distances m[t,c] = z_t.c_c - 0.5*||c_c||^2 (argmin d2 == argmax m)
    via a 2-matmul bf16 "packed split" per 512-code chunk:
      mm1: lhsT=[zH|zL] (128 rows)  rhs=[cH|cH]      -> zH.cH + zL.cH
      mm2: lhsT=[zH|1|1|0]          rhs=[cL|c2H|c2L|0] -> zH.cL - 0.5c2
    accumulated in fp32 PSUM; residual error ~2^-16 (argmin-exact in
    practice; z=zH+zL, c=cH+cL are bf16 high/low splits).
    K=128 keeps FWL weight loads fast.
  - argmax: ScalarE evacuates PSUM bank B; one DVE tensor_tensor(max)
    builds the 512-wide pair-max array g (+ its row max via max8), then
    max_index(g) yields the pair index j. Candidates j and j+512 are
    both gathered on-device (indirect DMA); the host resolves the 1-bit
    winner with two exact dot products per (token, codebook).
  - loss_vq == loss_commit == mean(min d2) from sum(z^2) (ScalarE
    square+accum) and sum(max m) (GPSIMD partition reduction).

kernel(z, codebooks) takes full inputs, returns
(out[B,D,H,W] f32, loss_vq f32, loss_commit f32, indices[T,N] int32)
matching reference.reference().
"""

import numpy as np

# ---- problem constants (hardcoded per harness rules) ----
B, D, H, W = 16, 512, 32, 32
N, M, DS = 8, 1024, 64
NCORES = 8
T = B * H * W                      # 16384 tokens
TL = T // NCORES                   # 2048 tokens per core
NT = TL // 128                     # 16 token tiles of 128
MH = M // 2                        # 512 (half the codebook)


def _rep_range(repeat):
    for _ in range(repeat):
        yield from range(N)


def _build_graph(repeat=1, variant="full"):
    """variant: 'mm' | 'max' | 'maxidx' | 'full' — progressively larger
    subsets of the pipeline (benchmarking aid)."""
    import concourse.bacc as bacc
    import concourse.bass as bass
    import concourse.mybir as mybir
    from concourse.bass_isa import ReduceOp
    from concourse.tile import TileContext

    lvl = ["mm", "max", "maxidx", "full"].index(variant)

    fp32 = mybir.dt.float32
    bf16 = mybir.dt.bfloat16
    u32 = mybir.dt.uint32

    nc = bacc.Bacc("TRN2", target_bir_lowering=False, debug=False)

    za = nc.declare_dram_parameter("za", [N, 128, TL], bf16, isOutput=False)
    zb = nc.declare_dram_parameter("zb", [N, 66, TL], bf16, isOutput=False)
    ca = nc.declare_dram_parameter("ca", [N, 128, M], bf16, isOutput=False)
    cb = nc.declare_dram_parameter("cb", [N, 66, M], bf16, isOutput=False)
    # paired gather table: row j = [code_j | code_{j+256} | code_{j+512} |
    # code_{j+768}] (1KB rows: one gather per tile fetches all 4 candidates)
    MQ = M // 4
    cfp = [
        nc.declare_dram_parameter(f"cfp{n}", [MQ, 4 * DS], fp32, isOutput=False)
        for n in range(N)
    ]
    zq = nc.declare_dram_parameter("zq", [N, 128, NT, 4 * DS], fp32, isOutput=True)
    idxp = nc.declare_dram_parameter("idxp", [128, N * NT * 8], u32, isOutput=True)
    lossp = nc.declare_dram_parameter("lossp", [1, 2], fp32, isOutput=True)

    with TileContext(nc) as tc:
        with (
            tc.tile_pool(name="cbp", bufs=2) as cbp,
            tc.tile_pool(name="zp", bufs=2) as zp,
            tc.tile_pool(name="ps", bufs=4, space="PSUM") as psp,
            tc.tile_pool(name="gp", bufs=3) as gp,
            tc.tile_pool(name="ixp", bufs=16) as ixp,
            tc.tile_pool(name="zqp", bufs=2) as zqp,
            tc.tile_pool(name="persist", bufs=1) as pp,
            tc.tile_pool(name="sq", bufs=2) as sqp,
            tc.tile_pool(name="fin", bufs=1) as fin,
        ):
            ix_all = pp.tile([128, N * NT * 8], u32)
            mx_all = pp.tile([128, N * NT], fp32)
            vdum = pp.tile([128, 1], fp32)
            zsq = pp.tile([128, N], fp32)
            nc.vector.memset(zsq[:], 0.0)
            if lvl < 3:  # benchmark variants leave parts unwritten
                nc.vector.memset(ix_all[:], 0)
                nc.vector.memset(mx_all[:], 0.0)

            for n in _rep_range(repeat):
                zat = zp.tile([128, TL], bf16, tag="za")
                nc.sync.dma_start(zat[:], za[n])
                zbt = zp.tile([66, TL], bf16, tag="zb")
                nc.sync.dma_start(zbt[:], zb[n])
                cat = cbp.tile([128, M], bf16, tag="ca")
                nc.sync.dma_start(cat[:], ca[n])
                cbt = cbp.tile([66, M], bf16, tag="cb")
                nc.sync.dma_start(cbt[:], cb[n])

                # z^2 loss term: sum(zH^2)+sum(zL^2) == sum(z^2) up to the
                # negligible (~5e-7 rel) 2*zH.zL cross term
                junk = sqp.tile([128, TL], fp32, tag="junk")
                nc.scalar.activation(
                    junk[:],
                    zat[:],
                    mybir.ActivationFunctionType.Square,
                    accum_out=zsq[:, n : n + 1],
                )

                for t in range(NT):
                    k = n * NT + t
                    sl = slice(t * 128, (t + 1) * 128)
                    dm = psp.tile([128, M], fp32)
                    for c0, c1 in ((0, MH), (MH, M)):
                        nc.tensor.matmul(
                            dm[:, c0:c1], lhsT=zat[:, sl], rhs=cat[:, c0:c1],
                            start=True, stop=False,
                        )
                        nc.tensor.matmul(
                            dm[:, c0:c1], lhsT=zbt[:, sl], rhs=cbt[:, c0:c1],
                            start=False, stop=True,
                        )
                    if lvl < 1:
                        continue
                    # ScalarE evacuates bank B; two DVE pair-max passes
                    # quarter the scan width (1024 -> 256)
                    evb = gp.tile([128, MH], fp32, tag="evb")
                    nc.scalar.copy(evb[:], dm[:, MH:M])
                    g = gp.tile([128, MH], fp32, tag="g")
                    nc.vector.tensor_tensor(
                        g[:], dm[:, 0:MH], evb[:], op=mybir.AluOpType.max
                    )
                    g2 = gp.tile([128, MH // 2], fp32, tag="g2")
                    nc.vector.tensor_tensor(
                        g2[:], g[:, 0 : MH // 2], g[:, MH // 2 : MH],
                        op=mybir.AluOpType.max,
                    )
                    # value-max via native tensor_scalar accumulate
                    # (2-port mode eligible, unlike max8)
                    nc.vector.tensor_scalar(
                        out=vdum.broadcast_to([128, MH // 2]),
                        in0=g2[:],
                        scalar1=1.0,
                        scalar2=None,
                        op0=mybir.AluOpType.mult,
                        op1=mybir.AluOpType.max,
                        accum_out=mx_all[:, k : k + 1],
                    )
                    if lvl < 2:
                        continue
                    # private per-tile index tile: avoids whole-tile WAR
                    # between the gathers (readers) and later max_index
                    # writes into a shared buffer
                    ixt = ixp.tile([128, 8], u32, tag="ixt")
                    nc.vector.max_index(
                        out=ixt[:],
                        in_max=mx_all[:, k : k + 1].to_broadcast([128, 8]),
                        in_values=g2[:],
                    )
                    nc.vector.tensor_copy(
                        ix_all[:, k * 8 : (k + 1) * 8], ixt[:]
                    )
                    if lvl < 3:
                        continue
                    # one gather per tile fetches both candidates (paired
                    # 512B rows); one offset per partition
                    if t == 0:
                        zgp = zqp.tile([128, NT, 4 * DS], fp32, tag="zgp")
                    nc.gpsimd.indirect_dma_start(
                        out=zgp[:, t, :],
                        out_offset=None,
                        in_=cfp[n][:],
                        in_offset=bass.IndirectOffsetOnAxis(
                            ap=ixt[:, 0:1], axis=0
                        ),
                    )
                if lvl < 3:
                    continue
                nc.sync.dma_start(zq[n], zgp[:])

            # ---- loss partials (partition reduction on GPSIMD) ----
            mxc = fin.tile([128, N * NT], fp32, tag="mxc")
            nc.vector.tensor_copy(mxc[:], mx_all[:])
            nc.gpsimd.partition_all_reduce(mxc[:], mxc[:], 128, ReduceOp.add)
            mxs = fin.tile([1, 1], fp32, tag="mxs")
            nc.vector.reduce_sum(mxs[:], mxc[0:1, :], axis=mybir.AxisListType.X)
            nc.gpsimd.partition_all_reduce(zsq[:], zsq[:], 128, ReduceOp.add)
            z2s = fin.tile([1, 1], fp32, tag="z2s")
            nc.vector.reduce_sum(z2s[:], zsq[0:1, :], axis=mybir.AxisListType.X)
            lb = fin.tile([1, 2], fp32, tag="lb")
            nc.vector.tensor_copy(lb[:, 0:1], z2s[:])
            nc.vector.tensor_copy(lb[:, 1:2], mxs[:])
            nc.sync.dma_start(lossp[:], lb[:])
            nc.sync.dma_start(idxp[:], ix_all[:])

    nc.compile()
    return nc


_NC_CACHE = {}


def _get_nc(repeat=1, variant="full"):
    key = (repeat, variant)
    if key not in _NC_CACHE:
        _NC_CACHE[key] = _build_graph(repeat, variant)
    return _NC_CACHE[key]


last_exec_time_ns = None
last_profile = None


def prep_in_maps(z, codebooks):
    import ml_dtypes

    bf16 = ml_dtypes.bfloat16
    f32 = np.float32

    # token t = b*(H*W) + h*W + w ; zt[n, d, t]
    zt = (
        z.reshape(B, N, DS, H * W)
        .transpose(1, 2, 0, 3)
        .reshape(N, DS, T)
    )
    c2 = (codebooks.astype(np.float64) ** 2).sum(-1).astype(f32)  # [N, M]
    cbt = codebooks.transpose(0, 2, 1)  # [N, DS, M]
    cH = cbt.astype(bf16)
    cL = (cbt - cH.astype(f32)).astype(bf16)
    mc2 = -0.5 * c2
    c2H = mc2.astype(bf16)
    c2L = (mc2 - c2H.astype(f32)).astype(bf16)

    # ca: [cH | cH] (128 rows); cb: [cL | c2H | c2L] (66 rows)
    ca = np.concatenate([cH, cH], axis=1)  # [N, 128, M]
    cbp = np.concatenate(
        [cL, c2H[:, None, :], c2L[:, None, :]], axis=1
    )  # [N, 66, M]
    ca = np.ascontiguousarray(ca)
    cbp = np.ascontiguousarray(cbp)

    ones_rows = np.ones((N, 2, TL), dtype=bf16)
    in_maps = []
    for i in range(NCORES):
        zs = zt[:, :, i * TL : (i + 1) * TL]  # [N, DS, TL]
        zH = zs.astype(bf16)
        zL = (zs - zH.astype(f32)).astype(bf16)
        za = np.ascontiguousarray(np.concatenate([zH, zL], axis=1))
        zb = np.ascontiguousarray(np.concatenate([zH, ones_rows], axis=1))
        m = {"za": za, "zb": zb, "ca": ca, "cb": cbp}
        MQ = M // 4
        for n in range(N):
            m[f"cfp{n}"] = np.ascontiguousarray(
                np.concatenate(
                    [codebooks[n, q * MQ : (q + 1) * MQ] for q in range(4)],
                    axis=1,
                )
            )
        in_maps.append(m)
    return in_maps


def kernel(z, codebooks):
    import os

    from concourse.bass_utils import run_bass_kernel_spmd

    trace = bool(int(os.environ.get("VQ_TRACE", "0")))

    z = np.ascontiguousarray(z, dtype=np.float32)
    codebooks = np.ascontiguousarray(codebooks, dtype=np.float32)

    in_maps = prep_in_maps(z, codebooks)
    nc = _get_nc()
    res = run_bass_kernel_spmd(
        nc, in_maps, core_ids=list(range(NCORES)), trace=trace
    )
    results = res.results
    global last_exec_time_ns, last_profile
    last_exec_time_ns = res.exec_time_ns
    last_profile = getattr(res, "profile_json", None)

    # ---- host-side unshard + pair resolution ----
    zt = (
        z.reshape(B, N, DS, H * W)
        .transpose(1, 2, 0, 3)
        .reshape(N, DS, T)
    )
    c2 = (codebooks.astype(np.float64) ** 2).sum(-1).astype(np.float32)

    out_tok = np.empty((T, N, DS), dtype=np.float32)
    indices = np.empty((T, N), dtype=np.int32)
    z2_sum = 0.0
    mx_sum = 0.0
    for i in range(NCORES):
        r = results[i]
        # device layout [N, 128, NT, 4*DS]; local token = t*128 + p
        gp_ = (
            np.asarray(r["zq"])
            .reshape(N, 128, NT, 4, DS)
            .transpose(0, 2, 1, 3, 4)
            .reshape(N, TL, 4, DS)
        )
        ixp = np.asarray(r["idxp"]).reshape(128, N, NT, 8)
        lp = np.asarray(r["lossp"]).reshape(2)
        z2_sum += float(lp[0])
        mx_sum += float(lp[1])

        MQ = M // 4
        j = (
            ixp[:, :, :, 0].transpose(1, 2, 0).reshape(N, TL).astype(np.int64)
        )  # [N, TL] quad index, token local = t*128 + p
        sl = slice(i * TL, (i + 1) * TL)
        zs = zt[:, :, sl].transpose(0, 2, 1)  # [N, TL, DS]
        # exact scores for the 4 candidates j + q*MQ, q=0..3
        c2q = np.stack(
            [
                np.take_along_axis(c2[:, q * MQ : (q + 1) * MQ], j, axis=1)
                for q in range(4)
            ],
            axis=2,
        )  # [N, TL, 4]
        mq = (
            np.einsum("ntd,ntqd->ntq", zs, gp_, optimize=True) - 0.5 * c2q
        )  # [N, TL, 4]
        pick = mq.argmax(axis=2)  # first occurrence -> lowest code index
        sel = np.take_along_axis(
            gp_, pick[..., None, None], axis=2
        ).squeeze(2)  # [N, TL, DS]
        out_tok[sl] = sel.transpose(1, 0, 2)
        indices[sl] = (j + pick * MQ).T.astype(np.int32)

    out = (
        out_tok.reshape(B, H * W, D)
        .transpose(0, 2, 1)
        .reshape(B, D, H, W)
    )
    loss = np.float32((z2_sum - 2.0 * mx_sum) / (N * T * DS))
    return out, loss, loss, indices


# revision 58
# speedup vs baseline: 1.0467x; 1.0467x over previous
"""DCVQ quantizer (vq_codebook) on 8 TRN2 NeuronCores.

Sharding (per spec hint): data-parallel over tokens (B*H*W), codebooks
replicated on every core; scalar loss partials summed on host.

Per core pipeline:
  - distances m[t,c] = z_t.c_c - 0.5*||c_c||^2 (argmin d2 == argmax m)
    via a 2-matmul bf16 "packed split" per 512-code chunk:
      mm1: lhsT=[zH|zL] (128 rows)  rhs=[cH|cH]      -> zH.cH + zL.cH
      mm2: lhsT=[zH|1|1|0]          rhs=[cL|c2H|c2L|0] -> zH.cL - 0.5c2
    accumulated in fp32 PSUM; residual error ~2^-16 (argmin-exact in
    practice; z=zH+zL, c=cH+cL are bf16 high/low splits).
    K=128 keeps FWL weight loads fast.
  - argmax: ScalarE evacuates PSUM bank B; one DVE tensor_tensor(max)
    builds the 512-wide pair-max array g (+ its row max via max8), then
    max_index(g) yields the pair index j. Candidates j and j+512 are
    both gathered on-device (indirect DMA); the host resolves the 1-bit
    winner with two exact dot products per (token, codebook).
  - loss_vq == loss_commit == mean(min d2) from sum(z^2) (ScalarE
    square+accum) and sum(max m) (GPSIMD partition reduction).

kernel(z, codebooks) takes full inputs, returns
(out[B,D,H,W] f32, loss_vq f32, loss_commit f32, indices[T,N] int32)
matching reference.reference().
"""

import numpy as np

# ---- problem constants (hardcoded per harness rules) ----
B, D, H, W = 16, 512, 32, 32
N, M, DS = 8, 1024, 64
NCORES = 8
T = B * H * W                      # 16384 tokens
TL = T // NCORES                   # 2048 tokens per core
NT = TL // 128                     # 16 token tiles of 128
MH = M // 2                        # 512 (half the codebook)


def _rep_range(repeat):
    for _ in range(repeat):
        yield from range(N)


def _build_graph(repeat=1, variant="full"):
    """variant: 'mm' | 'max' | 'maxidx' | 'full' — progressively larger
    subsets of the pipeline (benchmarking aid)."""
    import concourse.bacc as bacc
    import concourse.bass as bass
    import concourse.mybir as mybir
    from concourse.bass_isa import ReduceOp
    from concourse.tile import TileContext

    lvl = ["mm", "max", "maxidx", "full"].index(variant)

    fp32 = mybir.dt.float32
    bf16 = mybir.dt.bfloat16
    u32 = mybir.dt.uint32

    nc = bacc.Bacc("TRN2", target_bir_lowering=False, debug=False)

    za = nc.declare_dram_parameter("za", [N, 128, TL], bf16, isOutput=False)
    zb = nc.declare_dram_parameter("zb", [N, 66, TL], bf16, isOutput=False)
    ca = nc.declare_dram_parameter("ca", [N, 128, M], bf16, isOutput=False)
    cb = nc.declare_dram_parameter("cb", [N, 66, M], bf16, isOutput=False)
    # paired gather table: row j = [code_j | code_{j+256} | code_{j+512} |
    # code_{j+768}] (1KB rows: one gather per tile fetches all 4 candidates)
    MQ = M // 4
    cfp = [
        nc.declare_dram_parameter(f"cfp{n}", [MQ, 4 * DS], fp32, isOutput=False)
        for n in range(N)
    ]
    zq = nc.declare_dram_parameter("zq", [N, 128, NT, 4 * DS], fp32, isOutput=True)
    idxp = nc.declare_dram_parameter("idxp", [128, N * NT * 8], u32, isOutput=True)
    lossp = nc.declare_dram_parameter("lossp", [1, 2], fp32, isOutput=True)

    with TileContext(nc) as tc:
        with (
            tc.tile_pool(name="cbp", bufs=2) as cbp,
            tc.tile_pool(name="zp", bufs=2) as zp,
            tc.tile_pool(name="ps", bufs=4, space="PSUM") as psp,
            tc.tile_pool(name="gp", bufs=3) as gp,
            tc.tile_pool(name="ixp", bufs=16) as ixp,
            tc.tile_pool(name="zqp", bufs=2) as zqp,
            tc.tile_pool(name="persist", bufs=1) as pp,
            tc.tile_pool(name="sq", bufs=2) as sqp,
            tc.tile_pool(name="fin", bufs=1) as fin,
        ):
            ix_all = pp.tile([128, N * NT * 8], u32)
            mx_all = pp.tile([128, N * NT], fp32)
            vdum = pp.tile([128, 1], fp32)
            zsq = pp.tile([128, N], fp32)
            nc.vector.memset(zsq[:], 0.0)
            if lvl < 3:  # benchmark variants leave parts unwritten
                nc.vector.memset(ix_all[:], 0)
                nc.vector.memset(mx_all[:], 0.0)

            for n in _rep_range(repeat):
                zat = zp.tile([128, TL], bf16, tag="za")
                nc.sync.dma_start(zat[:], za[n])
                zbt = zp.tile([66, TL], bf16, tag="zb")
                nc.sync.dma_start(zbt[:], zb[n])
                cat = cbp.tile([128, M], bf16, tag="ca")
                nc.sync.dma_start(cat[:], ca[n])
                cbt = cbp.tile([66, M], bf16, tag="cb")
                nc.sync.dma_start(cbt[:], cb[n])

                # z^2 loss term: sum(zH^2)+sum(zL^2) == sum(z^2) up to the
                # negligible (~5e-7 rel) 2*zH.zL cross term
                junk = sqp.tile([128, TL], fp32, tag="junk")
                nc.scalar.activation(
                    junk[:],
                    zat[:],
                    mybir.ActivationFunctionType.Square,
                    accum_out=zsq[:, n : n + 1],
                )

                for t in range(NT):
                    k = n * NT + t
                    sl = slice(t * 128, (t + 1) * 128)
                    dm = psp.tile([128, M], fp32)
                    for c0, c1 in ((0, MH), (MH, M)):
                        nc.tensor.matmul(
                            dm[:, c0:c1], lhsT=zat[:, sl], rhs=cat[:, c0:c1],
                            start=True, stop=False,
                        )
                        nc.tensor.matmul(
                            dm[:, c0:c1], lhsT=zbt[:, sl], rhs=cbt[:, c0:c1],
                            start=False, stop=True,
                        )
                    if lvl < 1:
                        continue
                    # ScalarE evacuates bank B; two DVE pair-max passes
                    # quarter the scan width (1024 -> 256)
                    evb = gp.tile([128, MH], fp32, tag="evb")
                    nc.scalar.copy(evb[:], dm[:, MH:M])
                    g = gp.tile([128, MH], fp32, tag="g")
                    nc.vector.tensor_tensor(
                        g[:], dm[:, 0:MH], evb[:], op=mybir.AluOpType.max
                    )
                    g2 = gp.tile([128, MH // 2], fp32, tag="g2")
                    nc.vector.tensor_tensor(
                        g2[:], g[:, 0 : MH // 2], g[:, MH // 2 : MH],
                        op=mybir.AluOpType.max,
                    )
                    # value-max via native tensor_scalar accumulate
                    # (2-port mode eligible, unlike max8)
                    g3 = gp.tile([128, MH // 2], fp32, tag="g3")
                    nc.vector.tensor_scalar(
                        out=g3[:],
                        in0=g2[:],
                        scalar1=1.0,
                        scalar2=None,
                        op0=mybir.AluOpType.mult,
                        op1=mybir.AluOpType.max,
                        accum_out=mx_all[:, k : k + 1],
                    )
                    if lvl < 2:
                        continue
                    ixt = ix_all[:, k * 8 : (k + 1) * 8]
                    nc.vector.max_index(
                        out=ixt,
                        in_max=mx_all[:, k : k + 1].to_broadcast([128, 8]),
                        in_values=g2[:],
                    )
                    if lvl < 3:
                        continue
                    # one gather per tile fetches both candidates (paired
                    # 512B rows); one offset per partition
                    if t == 0:
                        zgp = zqp.tile([128, NT, 4 * DS], fp32, tag="zgp")
                    nc.gpsimd.indirect_dma_start(
                        out=zgp[:, t, :],
                        out_offset=None,
                        in_=cfp[n][:],
                        in_offset=bass.IndirectOffsetOnAxis(
                            ap=ixt[:, 0:1], axis=0
                        ),
                    )
                if lvl < 3:
                    continue
                nc.sync.dma_start(zq[n], zgp[:])

            # ---- loss partials (partition reduction on GPSIMD) ----
            mxc = fin.tile([128, N * NT], fp32, tag="mxc")
            nc.vector.tensor_copy(mxc[:], mx_all[:])
            nc.gpsimd.partition_all_reduce(mxc[:], mxc[:], 128, ReduceOp.add)
            mxs = fin.tile([1, 1], fp32, tag="mxs")
            nc.vector.reduce_sum(mxs[:], mxc[0:1, :], axis=mybir.AxisListType.X)
            nc.gpsimd.partition_all_reduce(zsq[:], zsq[:], 128, ReduceOp.add)
            z2s = fin.tile([1, 1], fp32, tag="z2s")
            nc.vector.reduce_sum(z2s[:], zsq[0:1, :], axis=mybir.AxisListType.X)
            lb = fin.tile([1, 2], fp32, tag="lb")
            nc.vector.tensor_copy(lb[:, 0:1], z2s[:])
            nc.vector.tensor_copy(lb[:, 1:2], mxs[:])
            nc.sync.dma_start(lossp[:], lb[:])
            nc.sync.dma_start(idxp[:], ix_all[:])

    nc.compile()
    return nc


_NC_CACHE = {}


def _get_nc(repeat=1, variant="full"):
    key = (repeat, variant)
    if key not in _NC_CACHE:
        _NC_CACHE[key] = _build_graph(repeat, variant)
    return _NC_CACHE[key]


last_exec_time_ns = None
last_profile = None


def prep_in_maps(z, codebooks):
    import ml_dtypes

    bf16 = ml_dtypes.bfloat16
    f32 = np.float32

    # token t = b*(H*W) + h*W + w ; zt[n, d, t]
    zt = (
        z.reshape(B, N, DS, H * W)
        .transpose(1, 2, 0, 3)
        .reshape(N, DS, T)
    )
    c2 = (codebooks.astype(np.float64) ** 2).sum(-1).astype(f32)  # [N, M]
    cbt = codebooks.transpose(0, 2, 1)  # [N, DS, M]
    cH = cbt.astype(bf16)
    cL = (cbt - cH.astype(f32)).astype(bf16)
    mc2 = -0.5 * c2
    c2H = mc2.astype(bf16)
    c2L = (mc2 - c2H.astype(f32)).astype(bf16)

    # ca: [cH | cH] (128 rows); cb: [cL | c2H | c2L] (66 rows)
    ca = np.concatenate([cH, cH], axis=1)  # [N, 128, M]
    cbp = np.concatenate(
        [cL, c2H[:, None, :], c2L[:, None, :]], axis=1
    )  # [N, 66, M]
    ca = np.ascontiguousarray(ca)
    cbp = np.ascontiguousarray(cbp)

    ones_rows = np.ones((N, 2, TL), dtype=bf16)
    in_maps = []
    for i in range(NCORES):
        zs = zt[:, :, i * TL : (i + 1) * TL]  # [N, DS, TL]
        zH = zs.astype(bf16)
        zL = (zs - zH.astype(f32)).astype(bf16)
        za = np.ascontiguousarray(np.concatenate([zH, zL], axis=1))
        zb = np.ascontiguousarray(np.concatenate([zH, ones_rows], axis=1))
        m = {"za": za, "zb": zb, "ca": ca, "cb": cbp}
        MQ = M // 4
        for n in range(N):
            m[f"cfp{n}"] = np.ascontiguousarray(
                np.concatenate(
                    [codebooks[n, q * MQ : (q + 1) * MQ] for q in range(4)],
                    axis=1,
                )
            )
        in_maps.append(m)
    return in_maps


def kernel(z, codebooks):
    import os

    from concourse.bass_utils import run_bass_kernel_spmd

    trace = bool(int(os.environ.get("VQ_TRACE", "0")))

    z = np.ascontiguousarray(z, dtype=np.float32)
    codebooks = np.ascontiguousarray(codebooks, dtype=np.float32)

    in_maps = prep_in_maps(z, codebooks)
    nc = _get_nc()
    res = run_bass_kernel_spmd(
        nc, in_maps, core_ids=list(range(NCORES)), trace=trace
    )
    results = res.results
    global last_exec_time_ns, last_profile
    last_exec_time_ns = res.exec_time_ns
    last_profile = getattr(res, "profile_json", None)

    # ---- host-side unshard + pair resolution ----
    zt = (
        z.reshape(B, N, DS, H * W)
        .transpose(1, 2, 0, 3)
        .reshape(N, DS, T)
    )
    c2 = (codebooks.astype(np.float64) ** 2).sum(-1).astype(np.float32)

    out_tok = np.empty((T, N, DS), dtype=np.float32)
    indices = np.empty((T, N), dtype=np.int32)
    z2_sum = 0.0
    mx_sum = 0.0
    for i in range(NCORES):
        r = results[i]
        # device layout [N, 128, NT, 4*DS]; local token = t*128 + p
        gp_ = (
            np.asarray(r["zq"])
            .reshape(N, 128, NT, 4, DS)
            .transpose(0, 2, 1, 3, 4)
            .reshape(N, TL, 4, DS)
        )
        ixp = np.asarray(r["idxp"]).reshape(128, N, NT, 8)
        lp = np.asarray(r["lossp"]).reshape(2)
        z2_sum += float(lp[0])
        mx_sum += float(lp[1])

        MQ = M // 4
        j = (
            ixp[:, :, :, 0].transpose(1, 2, 0).reshape(N, TL).astype(np.int64)
        )  # [N, TL] quad index, token local = t*128 + p
        sl = slice(i * TL, (i + 1) * TL)
        zs = zt[:, :, sl].transpose(0, 2, 1)  # [N, TL, DS]
        # exact scores for the 4 candidates j + q*MQ, q=0..3
        c2q = np.stack(
            [
                np.take_along_axis(c2[:, q * MQ : (q + 1) * MQ], j, axis=1)
                for q in range(4)
            ],
            axis=2,
        )  # [N, TL, 4]
        mq = (
            np.einsum("ntd,ntqd->ntq", zs, gp_, optimize=True) - 0.5 * c2q
        )  # [N, TL, 4]
        pick = mq.argmax(axis=2)  # first occurrence -> lowest code index
        sel = np.take_along_axis(
            gp_, pick[..., None, None], axis=2
        ).squeeze(2)  # [N, TL, DS]
        out_tok[sl] = sel.transpose(1, 0, 2)
        indices[sl] = (j + pick * MQ).T.astype(np.int32)

    out = (
        out_tok.reshape(B, H * W, D)
        .transpose(0, 2, 1)
        .reshape(B, D, H, W)
    )
    loss = np.float32((z2_sum - 2.0 * mx_sum) / (N * T * DS))
    return out, loss, loss, indices


# revision 59
# speedup vs baseline: 1.0623x; 1.0149x over previous
"""DCVQ quantizer (vq_codebook) on 8 TRN2 NeuronCores.

Sharding (per spec hint): data-parallel over tokens (B*H*W), codebooks
replicated on every core; scalar loss partials summed on host.

Per core pipeline:
  - distances m[t,c] = z_t.c_c - 0.5*||c_c||^2 (argmin d2 == argmax m)
    via a 2-matmul bf16 "packed split" per 512-code chunk:
      mm1: lhsT=[zH|zL] (128 rows)  rhs=[cH|cH]      -> zH.cH + zL.cH
      mm2: lhsT=[zH|1|1|0]          rhs=[cL|c2H|c2L|0] -> zH.cL - 0.5c2
    accumulated in fp32 PSUM; residual error ~2^-16 (argmin-exact in
    practice; z=zH+zL, c=cH+cL are bf16 high/low splits).
    K=128 keeps FWL weight loads fast.
  - argmax: ScalarE evacuates PSUM bank B; two DVE tensor_tensor(max)
    passes fold the 1024 scores to a 256-wide quad-max array g2 (pair
    reads use both DVE ports, 2 elem/cycle); a native tensor_scalar
    accumulate-max extracts the row max (exact, feeds the loss), and
    max_index(g2) yields the quad index j. The 4 candidate codes
    {j, j+256, j+512, j+768} live in one pre-paired 1KB row of the
    gather table, fetched by a single indirect DMA per tile; the host
    resolves the 2-bit winner with four exact dot products per
    (token, codebook).
  - loss_vq == loss_commit == mean(min d2) from sum(z^2) (ScalarE
    square+accum) and sum(max m) (GPSIMD partition reduction).

kernel(z, codebooks) takes full inputs, returns
(out[B,D,H,W] f32, loss_vq f32, loss_commit f32, indices[T,N] int32)
matching reference.reference().
"""

import numpy as np

# ---- problem constants (hardcoded per harness rules) ----
B, D, H, W = 16, 512, 32, 32
N, M, DS = 8, 1024, 64
NCORES = 8
T = B * H * W                      # 16384 tokens
TL = T // NCORES                   # 2048 tokens per core
NT = TL // 128                     # 16 token tiles of 128
MH = M // 2                        # 512 (half the codebook)


def _rep_range(repeat):
    for _ in range(repeat):
        yield from range(N)


def _build_graph(repeat=1, variant="full"):
    """variant: 'mm' | 'max' | 'maxidx' | 'full' — progressively larger
    subsets of the pipeline (benchmarking aid)."""
    import concourse.bacc as bacc
    import concourse.bass as bass
    import concourse.mybir as mybir
    from concourse.bass_isa import ReduceOp
    from concourse.tile import TileContext

    lvl = ["mm", "max", "maxidx", "full"].index(variant)

    fp32 = mybir.dt.float32
    bf16 = mybir.dt.bfloat16
    u32 = mybir.dt.uint32

    nc = bacc.Bacc("TRN2", target_bir_lowering=False, debug=False)

    za = nc.declare_dram_parameter("za", [N, 128, TL], bf16, isOutput=False)
    zb = nc.declare_dram_parameter("zb", [N, 66, TL], bf16, isOutput=False)
    ca = nc.declare_dram_parameter("ca", [N, 128, M], bf16, isOutput=False)
    cb = nc.declare_dram_parameter("cb", [N, 66, M], bf16, isOutput=False)
    # paired gather table: row j = [code_j | code_{j+256} | code_{j+512} |
    # code_{j+768}] (1KB rows: one gather per tile fetches all 4 candidates)
    MQ = M // 4
    cfp = [
        nc.declare_dram_parameter(f"cfp{n}", [MQ, 4 * DS], fp32, isOutput=False)
        for n in range(N)
    ]
    zq = nc.declare_dram_parameter("zq", [N, 128, NT, 4 * DS], fp32, isOutput=True)
    idxp = nc.declare_dram_parameter("idxp", [128, N * NT * 8], u32, isOutput=True)
    lossp = nc.declare_dram_parameter("lossp", [1, 2], fp32, isOutput=True)

    with TileContext(nc) as tc:
        with (
            tc.tile_pool(name="cbp", bufs=2) as cbp,
            tc.tile_pool(name="zp", bufs=2) as zp,
            tc.tile_pool(name="ps", bufs=4, space="PSUM") as psp,
            tc.tile_pool(name="gp", bufs=3) as gp,
            tc.tile_pool(name="zqp", bufs=2) as zqp,
            tc.tile_pool(name="persist", bufs=1) as pp,
            tc.tile_pool(name="sq", bufs=2) as sqp,
            tc.tile_pool(name="fin", bufs=1) as fin,
        ):
            ix_all = pp.tile([128, N * NT * 8], u32)
            mx_all = pp.tile([128, N * NT], fp32)
            zsq = pp.tile([128, N], fp32)
            nc.vector.memset(zsq[:], 0.0)
            if lvl < 3:  # benchmark variants leave parts unwritten
                nc.vector.memset(ix_all[:], 0)
                nc.vector.memset(mx_all[:], 0.0)

            for n in _rep_range(repeat):
                zat = zp.tile([128, TL], bf16, tag="za")
                nc.sync.dma_start(zat[:], za[n])
                zbt = zp.tile([66, TL], bf16, tag="zb")
                nc.sync.dma_start(zbt[:], zb[n])
                cat = cbp.tile([128, M], bf16, tag="ca")
                nc.sync.dma_start(cat[:], ca[n])
                cbt = cbp.tile([66, M], bf16, tag="cb")
                nc.sync.dma_start(cbt[:], cb[n])

                # z^2 loss term: sum(zH^2)+sum(zL^2) == sum(z^2) up to the
                # negligible (~5e-7 rel) 2*zH.zL cross term
                junk = sqp.tile([128, TL], fp32, tag="junk")
                nc.scalar.activation(
                    junk[:],
                    zat[:],
                    mybir.ActivationFunctionType.Square,
                    accum_out=zsq[:, n : n + 1],
                )

                for t in range(NT):
                    k = n * NT + t
                    sl = slice(t * 128, (t + 1) * 128)
                    dm = psp.tile([128, M], fp32)
                    for c0, c1 in ((0, MH), (MH, M)):
                        nc.tensor.matmul(
                            dm[:, c0:c1], lhsT=zat[:, sl], rhs=cat[:, c0:c1],
                            start=True, stop=False,
                        )
                        nc.tensor.matmul(
                            dm[:, c0:c1], lhsT=zbt[:, sl], rhs=cbt[:, c0:c1],
                            start=False, stop=True,
                        )
                    if lvl < 1:
                        continue
                    # ScalarE evacuates bank B; two DVE pair-max passes
                    # quarter the scan width (1024 -> 256)
                    evb = gp.tile([128, MH], fp32, tag="evb")
                    nc.scalar.copy(evb[:], dm[:, MH:M])
                    g = gp.tile([128, MH], fp32, tag="g")
                    nc.vector.tensor_tensor(
                        g[:], dm[:, 0:MH], evb[:], op=mybir.AluOpType.max
                    )
                    g2 = gp.tile([128, MH // 2], fp32, tag="g2")
                    nc.vector.tensor_tensor(
                        g2[:], g[:, 0 : MH // 2], g[:, MH // 2 : MH],
                        op=mybir.AluOpType.max,
                    )
                    # value-max via native tensor_scalar accumulate
                    # (2-port mode eligible, unlike max8)
                    g3 = gp.tile([128, MH // 2], fp32, tag="g3")
                    nc.vector.tensor_scalar(
                        out=g3[:],
                        in0=g2[:],
                        scalar1=1.0,
                        scalar2=None,
                        op0=mybir.AluOpType.mult,
                        op1=mybir.AluOpType.max,
                        accum_out=mx_all[:, k : k + 1],
                    )
                    if lvl < 2:
                        continue
                    ixt = ix_all[:, k * 8 : (k + 1) * 8]
                    nc.vector.max_index(
                        out=ixt,
                        in_max=mx_all[:, k : k + 1].to_broadcast([128, 8]),
                        in_values=g2[:],
                    )
                    if lvl < 3:
                        continue
                    # one gather per tile fetches both candidates (paired
                    # 512B rows); one offset per partition
                    if t == 0:
                        zgp = zqp.tile([128, NT, 4 * DS], fp32, tag="zgp")
                    nc.gpsimd.indirect_dma_start(
                        out=zgp[:, t, :],
                        out_offset=None,
                        in_=cfp[n][:],
                        in_offset=bass.IndirectOffsetOnAxis(
                            ap=ixt[:, 0:1], axis=0
                        ),
                    )
                if lvl < 3:
                    continue
                nc.sync.dma_start(zq[n], zgp[:])

            # ---- loss partials (partition reduction on GPSIMD) ----
            mxc = fin.tile([128, N * NT], fp32, tag="mxc")
            nc.vector.tensor_copy(mxc[:], mx_all[:])
            nc.gpsimd.partition_all_reduce(mxc[:], mxc[:], 128, ReduceOp.add)
            mxs = fin.tile([1, 1], fp32, tag="mxs")
            nc.vector.reduce_sum(mxs[:], mxc[0:1, :], axis=mybir.AxisListType.X)
            nc.gpsimd.partition_all_reduce(zsq[:], zsq[:], 128, ReduceOp.add)
            z2s = fin.tile([1, 1], fp32, tag="z2s")
            nc.vector.reduce_sum(z2s[:], zsq[0:1, :], axis=mybir.AxisListType.X)
            lb = fin.tile([1, 2], fp32, tag="lb")
            nc.vector.tensor_copy(lb[:, 0:1], z2s[:])
            nc.vector.tensor_copy(lb[:, 1:2], mxs[:])
            nc.sync.dma_start(lossp[:], lb[:])
            nc.sync.dma_start(idxp[:], ix_all[:])

    nc.compile()
    return nc


_NC_CACHE = {}


def _get_nc(repeat=1, variant="full"):
    key = (repeat, variant)
    if key not in _NC_CACHE:
        _NC_CACHE[key] = _build_graph(repeat, variant)
    return _NC_CACHE[key]


last_exec_time_ns = None
last_profile = None


def prep_in_maps(z, codebooks):
    import ml_dtypes

    bf16 = ml_dtypes.bfloat16
    f32 = np.float32

    # token t = b*(H*W) + h*W + w ; zt[n, d, t]
    zt = (
        z.reshape(B, N, DS, H * W)
        .transpose(1, 2, 0, 3)
        .reshape(N, DS, T)
    )
    c2 = (codebooks.astype(np.float64) ** 2).sum(-1).astype(f32)  # [N, M]
    cbt = codebooks.transpose(0, 2, 1)  # [N, DS, M]
    cH = cbt.astype(bf16)
    cL = (cbt - cH.astype(f32)).astype(bf16)
    mc2 = -0.5 * c2
    c2H = mc2.astype(bf16)
    c2L = (mc2 - c2H.astype(f32)).astype(bf16)

    # ca: [cH | cH] (128 rows); cb: [cL | c2H | c2L] (66 rows)
    ca = np.concatenate([cH, cH], axis=1)  # [N, 128, M]
    cbp = np.concatenate(
        [cL, c2H[:, None, :], c2L[:, None, :]], axis=1
    )  # [N, 66, M]
    ca = np.ascontiguousarray(ca)
    cbp = np.ascontiguousarray(cbp)

    ones_rows = np.ones((N, 2, TL), dtype=bf16)
    in_maps = []
    for i in range(NCORES):
        zs = zt[:, :, i * TL : (i + 1) * TL]  # [N, DS, TL]
        zH = zs.astype(bf16)
        zL = (zs - zH.astype(f32)).astype(bf16)
        za = np.ascontiguousarray(np.concatenate([zH, zL], axis=1))
        zb = np.ascontiguousarray(np.concatenate([zH, ones_rows], axis=1))
        m = {"za": za, "zb": zb, "ca": ca, "cb": cbp}
        MQ = M // 4
        for n in range(N):
            m[f"cfp{n}"] = np.ascontiguousarray(
                np.concatenate(
                    [codebooks[n, q * MQ : (q + 1) * MQ] for q in range(4)],
                    axis=1,
                )
            )
        in_maps.append(m)
    return in_maps


def kernel(z, codebooks):
    import os

    from concourse.bass_utils import run_bass_kernel_spmd

    trace = bool(int(os.environ.get("VQ_TRACE", "0")))

    z = np.ascontiguousarray(z, dtype=np.float32)
    codebooks = np.ascontiguousarray(codebooks, dtype=np.float32)

    in_maps = prep_in_maps(z, codebooks)
    nc = _get_nc()
    res = run_bass_kernel_spmd(
        nc, in_maps, core_ids=list(range(NCORES)), trace=trace
    )
    results = res.results
    global last_exec_time_ns, last_profile
    last_exec_time_ns = res.exec_time_ns
    last_profile = getattr(res, "profile_json", None)

    # ---- host-side unshard + pair resolution ----
    zt = (
        z.reshape(B, N, DS, H * W)
        .transpose(1, 2, 0, 3)
        .reshape(N, DS, T)
    )
    c2 = (codebooks.astype(np.float64) ** 2).sum(-1).astype(np.float32)

    out_tok = np.empty((T, N, DS), dtype=np.float32)
    indices = np.empty((T, N), dtype=np.int32)
    z2_sum = 0.0
    mx_sum = 0.0
    for i in range(NCORES):
        r = results[i]
        # device layout [N, 128, NT, 4*DS]; local token = t*128 + p
        gp_ = (
            np.asarray(r["zq"])
            .reshape(N, 128, NT, 4, DS)
            .transpose(0, 2, 1, 3, 4)
            .reshape(N, TL, 4, DS)
        )
        ixp = np.asarray(r["idxp"]).reshape(128, N, NT, 8)
        lp = np.asarray(r["lossp"]).reshape(2)
        z2_sum += float(lp[0])
        mx_sum += float(lp[1])

        MQ = M // 4
        j = (
            ixp[:, :, :, 0].transpose(1, 2, 0).reshape(N, TL).astype(np.int64)
        )  # [N, TL] quad index, token local = t*128 + p
        sl = slice(i * TL, (i + 1) * TL)
        zs = zt[:, :, sl].transpose(0, 2, 1)  # [N, TL, DS]
        # exact scores for the 4 candidates j + q*MQ, q=0..3
        c2q = np.stack(
            [
                np.take_along_axis(c2[:, q * MQ : (q + 1) * MQ], j, axis=1)
                for q in range(4)
            ],
            axis=2,
        )  # [N, TL, 4]
        mq = (
            np.einsum("ntd,ntqd->ntq", zs, gp_, optimize=True) - 0.5 * c2q
        )  # [N, TL, 4]
        pick = mq.argmax(axis=2)  # first occurrence -> lowest code index
        sel = np.take_along_axis(
            gp_, pick[..., None, None], axis=2
        ).squeeze(2)  # [N, TL, DS]
        out_tok[sl] = sel.transpose(1, 0, 2)
        indices[sl] = (j + pick * MQ).T.astype(np.int32)

    out = (
        out_tok.reshape(B, H * W, D)
        .transpose(0, 2, 1)
        .reshape(B, D, H, W)
    )
    loss = np.float32((z2_sum - 2.0 * mx_sum) / (N * T * DS))
    return out, loss, loss, indices
